# revision 1
# baseline (speedup 1.0000x reference)
"""Trainium2 Bass kernel for nn_DiffusionModel_56822417326086.

Causal multi-head self-attention block:
    qkv = x @ w_qkv ; split into 8 heads of 64
    e = (q @ k^T) * DH^-0.5 ; causal + key-padding mask ; a = softmax(e)
    o = a @ v ; y = o @ w_out + b_out ; y *= m

Sharding (8 cores, zero collectives):
    core c -> batch b = c // 2, head-quad q = c % 2 (heads 4q..4q+3).
    Each core computes q/k/v for its 4 heads over its whole batch, full
    causal attention for those heads, and the partial output projection
    y_partial = o[heads] @ w_out[head rows].  Host sums the two partials
    per batch (linear unshard), adds b_out, applies the query-side mask.

On-device layout notes:
  - scores are computed TRANSPOSED: sT[key, query] so that the A@V
    contraction (over keys) has keys on the partition dim.
  - softmax denominators come for free as a 65th "ones" column of V.
  - no max-subtraction in softmax: scores are O(1) here, exp is safe.
  - matmuls run as float32r (fp32 data on the fast PE path).
  - all matmul operands live at partition base 0 (base-64 operands fault
    on this runtime), so q/k are stored per-head at partitions 0-63.
  - all 4 heads of one key block share a 2-bank PSUM tile [128, 1024]
    so one ACT Exp op covers them (ACT per-op overhead is ~250 ns).
"""

import numpy as np
import ml_dtypes
from contextlib import ExitStack

B, T, D, H = 4, 2048, 512, 8
DH = D // H
SCALE = DH ** -0.5
NEG = -1.0e30
QC = 512           # query-chunk (free dim of score matmuls)
NQC = T // QC      # 8
KB = 128           # key-block (partition dim of score tiles)

_CACHE = {}


def _build_program():
    import concourse.mybir as mybir
    import concourse.tile as tile
    from concourse import bacc

    f32 = mybir.dt.float32
    f32r = mybir.dt.float32r
    bf16 = mybir.dt.bfloat16
    Exp = mybir.ActivationFunctionType.Exp

    nc = bacc.Bacc("TRN2", target_bir_lowering=False, debug=False)

    xT_d = nc.dram_tensor("xT", [D, T], bf16, kind="ExternalInput").ap()
    wq_d = nc.dram_tensor("wq2", [2, D, 128], bf16, kind="ExternalInput").ap()
    wk_d = nc.dram_tensor("wk2", [2, D, 128], bf16, kind="ExternalInput").ap()
    wv_d = nc.dram_tensor("wv4", [D, 256], bf16, kind="ExternalInput").ap()
    wo_d = nc.dram_tensor("wo4", [256, D], f32r, kind="ExternalInput").ap()
    dm_d = nc.dram_tensor("dm4", [4, 128, 1024], bf16, kind="ExternalInput").ap()
    mk_d = nc.dram_tensor("mkey", [T, 1], f32, kind="ExternalInput").ap()
    y_d = nc.dram_tensor("y", [T, D], f32, kind="ExternalOutput").ap()

    with tile.TileContext(nc) as tc, ExitStack() as ctx:
        consts = ctx.enter_context(tc.tile_pool(name="consts", bufs=1))
        work = ctx.enter_context(tc.tile_pool(name="work", bufs=2))
        ps_big = ctx.enter_context(tc.tile_pool(name="psb", bufs=3, space="PSUM"))
        ps_o = ctx.enter_context(tc.tile_pool(name="pso", bufs=1, space="PSUM"))

        # ---- persistent tiles ----------------------------------------------
        # packed q^T/k^T: partitions 0-63 = head A of pair, 64-127 = head B
        qT2 = consts.tile([128, 2, T], f32r)
        kT2 = consts.tile([128, 2, T], f32r)
        vsb = consts.tile([128, 16, 4, 65], bf16)
        wo = consts.tile([128, 2, D], f32r)
        mk = consts.tile([128, 16], f32)
        ones41 = consts.tile([128, 4, 1], f32)
        oUA = consts.tile([64, 2, T], f32)
        oUB = consts.tile([64, 2, T], f32)
        sums_stage = consts.tile([36, 1024], f32)   # p0 rows 0-3, p1 rows 32-35
        recips_f = consts.tile([36, 1024], f32)
        recips = consts.tile([36, 1024], f32r)
        ones64 = consts.tile([1, 64], f32)
        ones64r = consts.tile([1, 64], f32r)
        oTn2 = consts.tile([128, 2, T], f32r)

        nc.vector.memset(ones41[:], 1.0)
        nc.vector.memset(ones64[:], 1.0)
        nc.vector.tensor_copy(ones64r[:], ones64[:])
        warm = consts.tile([1, 512], f32r)
        nc.vector.tensor_copy(warm[0:1, 0:64], ones64[:])
        for _ in range(40):
            wps = ps_big.tile([64, 512], f32, tag="scores")
            nc.tensor.matmul(wps[:], ones64r[:], warm[:], start=True, stop=True)
        for p in range(2):
            nc.sync.dma_start(wo[:, p, :], wo_d[p * 128:(p + 1) * 128, :])
        for rc in range(16):
            nc.sync.dma_start(mk[:, rc:rc + 1], mk_d[rc * 128:(rc + 1) * 128, :])

        # ---- qkv projection (phase-scoped SBUF pool) ------------------------
        with tc.tile_pool(name="qkvp", bufs=1) as qp:
            wq = qp.tile([128, 2, 4, 128], bf16)
            wk = qp.tile([128, 2, 4, 128], bf16)
            wv = qp.tile([128, 4, 256], bf16)
            for p in range(2):
                for kc in range(4):
                    nc.gpsimd.dma_start(wq[:, p, kc, :],
                                        wq_d[p, kc * 128:(kc + 1) * 128, :])
                    nc.scalar.dma_start(wk[:, p, kc, :],
                                        wk_d[p, kc * 128:(kc + 1) * 128, :])
            for kc in range(4):
                nc.sync.dma_start(wv[:, kc, :], wv_d[kc * 128:(kc + 1) * 128, :])
            xT = qp.tile([128, 4, T], bf16)
            # column-major sub-chunks so the first matmul group's inputs land
            # quickly instead of after the whole 4 MB of x
            _eng = [nc.sync, nc.gpsimd, nc.scalar, nc.gpsimd]
            for rc4 in range(4):
                for kc in range(4):
                    _eng[kc].dma_start(
                        xT[:, kc, rc4 * 512:(rc4 + 1) * 512],
                        xT_d[kc * 128:(kc + 1) * 128, rc4 * 512:(rc4 + 1) * 512])

            for p in range(2):
                for rc4 in range(4):
                    sl = slice(rc4 * 512, (rc4 + 1) * 512)
                    psq = ps_big.tile([128, 512], f32, tag="scores")
                    psk = ps_big.tile([128, 512], f32, tag="scores")
                    for kc in range(4):
                        nc.tensor.matmul(psq[:], wq[:, p, kc, :], xT[:, kc, sl],
                                         start=kc == 0, stop=kc == 3)
                        nc.tensor.matmul(psk[:], wk[:, p, kc, :], xT[:, kc, sl],
                                         start=kc == 0, stop=kc == 3)
                    nc.vector.tensor_copy(qT2[:, p, sl], psq[:])
                    nc.vector.tensor_copy(kT2[:, p, sl], psk[:])

            for rc in range(16):
                psv = ps_big.tile([128, 4, 64], f32, tag="scores")
                for kc in range(4):
                    nc.tensor.matmul(psv[:], xT[:, kc, rc * 128:(rc + 1) * 128],
                                     wv[:, kc, :], start=kc == 0, stop=kc == 3)
                nc.vector.tensor_scalar_mul(vsb[:, rc, :, 0:64], psv[:],
                                            mk[:, rc:rc + 1])
                nc.vector.tensor_scalar_mul(vsb[:, rc, :, 64:65], ones41[:],
                                            mk[:, rc:rc + 1])

        # ---- attention (pair-major; 2 heads per 2-bank score tile) ----------
        with tc.tile_pool(name="attp", bufs=1) as ap_, \
             tc.tile_pool(name="exp", bufs=4) as exp_pool:
            dm = ap_.tile([128, 4, 1024], bf16)
            for v_ in range(4):
                nc.sync.dma_start(dm[:, v_, :], dm_d[v_])
            for p in range(2):
                for qc in range(NQC):
                    nkb = 4 * (qc + 1)
                    qsl = slice(qc * QC, (qc + 1) * QC)
                    oA = ps_o.tile([128, 512], f32, tag="oA")
                    oB = ps_o.tile([128, 512], f32, tag="oB")
                    avq = []
                    for kb in range(nkb):
                        ksl = slice(kb * KB, (kb + 1) * KB)
                        sps = ps_big.tile([128, 1024], f32, tag="scores")
                        # row-tiled pair: K=64 each, concurrent in the array;
                        # outputs land in DIFFERENT PSUM banks (same-bank
                        # dual-write faults the exec unit)
                        nc.tensor.matmul(sps[:, 0:512], kT2[0:64, p, ksl],
                                         qT2[0:64, p, qsl], start=True, stop=True,
                                         tile_position=(0, 0))
                        nc.tensor.matmul(sps[:, 512:1024], kT2[64:128, p, ksl],
                                         qT2[64:128, p, qsl], start=True, stop=True,
                                         tile_position=(64, 0))
                        ex = exp_pool.tile([128, 1024], bf16, tag="exp")
                        nc.scalar.activation(ex[:], sps[:], Exp, scale=SCALE)
                        if kb >= nkb - 4:
                            nc.vector.tensor_mul(ex[:], ex[:],
                                                 dm[:, kb - (nkb - 4), :])
                        avq.append((kb, ex))
                        if len(avq) > 1:
                            _em(nc, avq.pop(0), oA, oB, vsb, p, nkb)
                    _em(nc, avq.pop(0), oA, oB, vsb, p, nkb)

                    scr = work.tile([128, 1024], f32, tag="sumscr")
                    nc.vector.tensor_copy(scr[64:65, 0:512], oA[64:65, :])
                    nc.vector.tensor_copy(scr[64:65, 512:1024], oB[64:65, :])
                    idx = p * 32 + qc
                    nc.sync.dma_start(sums_stage[idx:idx + 1, :], scr[64:65, :])
                    nc.vector.tensor_copy(oUA[:, p, qsl], oA[0:64, :])
                    nc.vector.tensor_copy(oUB[:, p, qsl], oB[0:64, :])

        # ---- normalize + output projection ---------------------------------
        # reciprocal is split per pair: p0's normalization only depends on
        # p0's sums, so the scheduler can run it under p1's attention.
        for p in range(2):
            nc.vector.reciprocal(recips_f[p * 32:p * 32 + 4, :],
                                 sums_stage[p * 32:p * 32 + 4, :])
            nc.vector.tensor_copy(recips[p * 32:p * 32 + 4, :],
                                  recips_f[p * 32:p * 32 + 4, :])
        for qc in range(NQC):
            qsl = slice(qc * QC, (qc + 1) * QC)
            for p in range(2):
                idx = p * 32 + qc
                rec = work.tile([1, 1024], f32r, tag="rec")
                nc.sync.dma_start(rec[:], recips[idx:idx + 1, :])
                bcA = ps_big.tile([64, 512], f32, tag="scores")
                nc.tensor.matmul(bcA[:], ones64r[:], rec[0:1, 0:512],
                                 start=True, stop=True)
                nc.vector.tensor_mul(oTn2[0:64, p, qsl], oUA[:, p, qsl], bcA[:])
                bcB = ps_big.tile([64, 512], f32, tag="scores")
                nc.tensor.matmul(bcB[:], ones64r[:], rec[0:1, 512:1024],
                                 start=True, stop=True)
                scrB = work.tile([64, 512], f32r, tag="scrB")
                nc.vector.tensor_mul(scrB[:], oUB[:, p, qsl], bcB[:])
                # partition shift 0-63 -> 64-127 (DVE lanes are partition-locked)
                nc.sync.dma_start(oTn2[64:128, p, qsl], scrB[:])
            for rc in range(4 * qc, 4 * qc + 4):
                rsl = slice(rc * 128, (rc + 1) * 128)
                psy = ps_big.tile([128, 512], f32, tag="scores")
                for p in range(2):
                    nc.tensor.matmul(psy[:], oTn2[:, p, rsl], wo[:, p, :],
                                     start=p == 0, stop=p == 1)
                yt = work.tile([128, 512], f32, tag="ysb")
                nc.vector.tensor_copy(yt[:], psy[:])
                nc.sync.dma_start(y_d[rsl, :], yt[:])

    nc.compile()
    return nc


def _em(nc, item, oA, oB, vsb, p, nkb):
    """Emit the deferred A@V accumulations for one key block (one pair)."""
    kb, ex = item
    nc.tensor.matmul(oA[0:65, :], vsb[:, kb, 2 * p, :], ex[:, 0:512],
                     start=kb == 0, stop=kb == nkb - 1)
    nc.tensor.matmul(oB[0:65, :], vsb[:, kb, 2 * p + 1, :], ex[:, 512:1024],
                     start=kb == 0, stop=kb == nkb - 1)


def _diag_masks():
    i = np.arange(QC)[None, :]
    j = np.arange(KB)[:, None]
    out = []
    for v in range(4):
        mv = np.where(i >= j + v * KB, 1.0, 0.0).astype(np.float32)
        out.append(np.tile(mv, (1, 2)).copy())
    return out


def _prep_inputs(x, m, w_qkv, w_out):
    """Per-core input maps for SPMD dispatch."""
    dm4 = np.stack(_diag_masks()).astype(ml_dtypes.bfloat16)
    wq_full = w_qkv[:, 0:D]
    wk_full = w_qkv[:, D:2 * D]
    wv_full = w_qkv[:, 2 * D:3 * D]
    in_maps = []
    for c in range(8):
        b, q = c // 2, c % 2
        hsl = slice(4 * q * DH, (4 * q + 4) * DH)
        wq2 = np.stack([
            np.concatenate([wq_full[:, (4 * q + 2 * p) * DH:(4 * q + 2 * p + 1) * DH],
                            wq_full[:, (4 * q + 2 * p + 1) * DH:(4 * q + 2 * p + 2) * DH]],
                           axis=1)
            for p in range(2)])
        wk2 = np.stack([
            np.concatenate([wk_full[:, (4 * q + 2 * p) * DH:(4 * q + 2 * p + 1) * DH],
                            wk_full[:, (4 * q + 2 * p + 1) * DH:(4 * q + 2 * p + 2) * DH]],
                           axis=1)
            for p in range(2)])
        in_maps.append({
            "xT": np.ascontiguousarray(x[b].T).astype(ml_dtypes.bfloat16),
            "wq2": np.ascontiguousarray(wq2).astype(ml_dtypes.bfloat16),
            "wk2": np.ascontiguousarray(wk2).astype(ml_dtypes.bfloat16),
            "wv4": np.ascontiguousarray(wv_full[:, hsl]).astype(ml_dtypes.bfloat16),
            "wo4": np.ascontiguousarray(w_out[hsl, :]).astype(np.float32),
            "dm4": dm4,
            "mkey": np.ascontiguousarray((m[b] != 0).astype(np.float32)[:, None]),
        })
    return in_maps


def _execute(inputs, trace=False):
    from concourse.bass_utils import run_bass_kernel_spmd

    if "nc" not in _CACHE:
        _CACHE["nc"] = _build_program()
    nc = _CACHE["nc"]

    x = np.asarray(inputs["x"], np.float32)
    m = np.asarray(inputs["m"], np.float32)
    w_qkv = np.asarray(inputs["w_qkv"], np.float32)
    w_out = np.asarray(inputs["w_out"], np.float32)
    b_out = np.asarray(inputs["b_out"], np.float32)

    in_maps = _prep_inputs(x, m, w_qkv, w_out)
    res = run_bass_kernel_spmd(nc, in_maps, core_ids=list(range(8)), trace=trace)

    y = np.empty((B, T, D), np.float32)
    for b in range(B):
        y[b] = res.results[2 * b]["y"] + res.results[2 * b + 1]["y"]
    y += b_out[None, None, :]
    y *= m[..., None]
    return y, res


def kernel(**inputs) -> np.ndarray:
    y, _ = _execute(inputs, trace=False)
    return y



# revision 4
# speedup vs baseline: 1.3431x; 1.3431x over previous
"""Trainium2 Bass kernel for nn_DiffusionModel_56822417326086.

Causal multi-head self-attention block:
    qkv = x @ w_qkv ; split into 8 heads of 64
    e = (q @ k^T) * DH^-0.5 ; causal + key-padding mask ; a = softmax(e)
    o = a @ v ; y = o @ w_out + b_out ; y *= m

Sharding (8 cores, zero collectives):
    core c -> batch b = c // 2, head-quad q = c % 2 (heads 4q..4q+3).
    Each core computes q/k/v for its 4 heads over its whole batch, full
    causal attention for those heads, and the partial output projection
    y_partial = o[heads] @ w_out[head rows].  Host sums the two partials
    per batch (linear unshard), adds b_out, applies the query-side mask.

v2: software-pipelined single-pass schedule.  The ACT engine's Exp
stream (80 ops x ~1us on [128,1024] tiles) is the hard floor (~80us);
everything else (qkv projection, score matmuls, A@V, normalization,
output projection) is emitted interleaved between attention blocks so
the in-order engine FIFOs overlap it under the Exp stream.

On-device layout notes:
  - scores are computed TRANSPOSED: sT[key, query] so that the A@V
    contraction (over keys) has keys on the partition dim.
  - softmax denominators come for free as a 65th "ones" column of V.
  - no max-subtraction in softmax: scores are O(1) here, exp is safe.
  - all 4 heads of one key block share a 2-bank PSUM tile [128, 1024]
    so one ACT Exp op covers them (ACT per-op overhead is ~290 ns).
  - A@V is deferred one block so the PE never head-of-line blocks on
    the exp -> mask chain.
  - o-tiles are evicted eagerly (one [65,1024] copy) so PSUM o-pool is
    a single buffer; normalization runs from SBUF one qc behind.
  - reciprocal runs on a DMA-reshaped [16,64] tile (DVE reciprocal is
    8 cyc/elem along the FREE dim only - [1,1024] rows cost 8.5us).
  - masks are applied post-exp on restricted column ranges only.
"""

import numpy as np
import ml_dtypes
from contextlib import ExitStack

B, T, D, H = 4, 2048, 512, 8
DH = D // H
SCALE = DH ** -0.5
QC = 512           # query-chunk (free dim of score matmuls)
NQC = T // QC      # 4
KB = 128           # key-block (partition dim of score tiles)

_CACHE = {}


def _build_program():
    import concourse.mybir as mybir
    import concourse.tile as tile
    from concourse import bacc

    f32 = mybir.dt.float32
    f32r = mybir.dt.float32r
    bf16 = mybir.dt.bfloat16
    Exp = mybir.ActivationFunctionType.Exp

    nc = bacc.Bacc("TRN2", target_bir_lowering=False, debug=False)

    xT_d = nc.dram_tensor("xT", [D, T], bf16, kind="ExternalInput").ap()
    wq_d = nc.dram_tensor("wq2", [2, D, 128], bf16, kind="ExternalInput").ap()
    wk_d = nc.dram_tensor("wk2", [2, D, 128], bf16, kind="ExternalInput").ap()
    wv_d = nc.dram_tensor("wv4", [D, 256], bf16, kind="ExternalInput").ap()
    wo_d = nc.dram_tensor("wo4", [256, D], bf16, kind="ExternalInput").ap()
    dm_d = nc.dram_tensor("dm4", [4, 128, 1024], bf16, kind="ExternalInput").ap()
    mk_d = nc.dram_tensor("mkey", [T, 1], f32, kind="ExternalInput").ap()
    y_d = nc.dram_tensor("y", [T, D], bf16, kind="ExternalOutput").ap()

    with tile.TileContext(nc) as tc, ExitStack() as ctx:
        consts = ctx.enter_context(tc.tile_pool(name="consts", bufs=1))
        ps_sc = ctx.enter_context(tc.tile_pool(name="pssc", bufs=2, space="PSUM"))
        ps_o = ctx.enter_context(tc.tile_pool(name="pso", bufs=1, space="PSUM"))
        ps_m = ctx.enter_context(tc.tile_pool(name="psm", bufs=2, space="PSUM"))
        exp_pool = ctx.enter_context(tc.tile_pool(name="exp", bufs=4))
        work = ctx.enter_context(tc.tile_pool(name="work", bufs=2))
        ou_pool = ctx.enter_context(tc.tile_pool(name="ou", bufs=2))
        rr_pool = ctx.enter_context(tc.tile_pool(name="rr", bufs=2))
        rec_pool = ctx.enter_context(tc.tile_pool(name="rec", bufs=2))
        bcs_pool = ctx.enter_context(tc.tile_pool(name="bcs", bufs=2))
        yt_pool = ctx.enter_context(tc.tile_pool(name="yt", bufs=2))

        # ---- persistent tiles ----------------------------------------------
        # packed q^T/k^T: partitions 0-63 = head A of pair, 64-127 = head B
        qT2 = consts.tile([128, 2, T], f32r)
        kT2 = consts.tile([128, 2, T], f32r)
        vsb = consts.tile([128, 16, 4, 65], bf16)
        wo = consts.tile([128, 2, D], bf16)
        mk = consts.tile([128, 16], f32)
        ones41 = consts.tile([128, 4, 1], f32)
        oTn2 = consts.tile([128, 2, T], bf16)
        ones64 = consts.tile([1, 64], f32)
        ones64r = consts.tile([1, 64], f32r)
        dm = consts.tile([128, 4, 1024], bf16)
        wq = consts.tile([128, 2, 4, 128], bf16)
        wk = consts.tile([128, 2, 4, 128], bf16)
        wv = consts.tile([128, 4, 256], bf16)
        xT = consts.tile([128, 4, T], bf16)
        junk = consts.tile([128, 64], f32)

        nc.vector.memset(ones41[:], 1.0)
        nc.vector.memset(ones64[:], 1.0)
        nc.vector.tensor_copy(ones64r[:], ones64[:])
        nc.vector.memset(junk[:], 0.0)

        # ---- input DMAs (ordered so the first work's inputs land first) ----
        # first q/k matmuls need wq/wk p=0 + xT[:, :, 0:512]
        for kc in range(4):
            nc.sync.dma_start(wq[:, 0, kc, :], wq_d[0, kc * 128:(kc + 1) * 128, :])
            nc.gpsimd.dma_start(wk[:, 0, kc, :], wk_d[0, kc * 128:(kc + 1) * 128, :])
        _eng = [nc.sync, nc.gpsimd, nc.sync, nc.gpsimd]
        for rc4 in range(4):
            for kc in range(4):
                _eng[kc].dma_start(
                    xT[:, kc, rc4 * 512:(rc4 + 1) * 512],
                    xT_d[kc * 128:(kc + 1) * 128, rc4 * 512:(rc4 + 1) * 512])
            if rc4 == 0:
                # v weights + masks right after the first x chunk
                for kc in range(4):
                    nc.sync.dma_start(wv[:, kc, :], wv_d[kc * 128:(kc + 1) * 128, :])
                for rc in range(16):
                    nc.gpsimd.dma_start(mk[:, rc:rc + 1],
                                        mk_d[rc * 128:(rc + 1) * 128, :])
                for v_ in range(4):
                    nc.sync.dma_start(dm[:, v_, :], dm_d[v_])
            if rc4 == 1:
                for kc in range(4):
                    nc.gpsimd.dma_start(wq[:, 1, kc, :],
                                        wq_d[1, kc * 128:(kc + 1) * 128, :])
                    nc.sync.dma_start(wk[:, 1, kc, :],
                                      wk_d[1, kc * 128:(kc + 1) * 128, :])
            if rc4 == 2:
                for p_ in range(2):
                    nc.gpsimd.dma_start(wo[:, p_, :], wo_d[p_ * 128:(p_ + 1) * 128, :])

        # ---- early ACT table load + PE warmup ------------------------------
        wex = exp_pool.tile([128, 64], bf16, tag="ex")
        nc.scalar.activation(wex[:], junk[:], Exp, scale=SCALE)
        warm = consts.tile([1, 512], f32r)
        nc.vector.tensor_copy(warm[0:1, 0:64], ones64[:])
        for _ in range(8):
            wps = ps_m.tile([64, 512], f32, tag="misc")
            nc.tensor.matmul(wps[:], ones64r[:], warm[:], start=True, stop=True)

        # ---- filler emission machinery -------------------------------------
        # Small units of non-attention work woven between attention blocks so
        # the PE/DVE FIFOs stay busy under the ACT exp stream.
        fillers = []

        def qk_unit(p, rc4, which):
            def emit():
                sl = slice(rc4 * 512, (rc4 + 1) * 512)
                ps = ps_m.tile([128, 512], f32, tag="misc")
                w_ = wq if which == 0 else wk
                dst = qT2 if which == 0 else kT2
                for kc in range(4):
                    nc.tensor.matmul(ps[:], w_[:, p, kc, :], xT[:, kc, sl],
                                     start=kc == 0, stop=kc == 3)
                nc.vector.tensor_copy(dst[:, p, sl], ps[:])
            return emit

        def v_unit(rc):
            def emit():
                psv = ps_m.tile([128, 4, 64], f32, tag="misc")
                for kc in range(4):
                    nc.tensor.matmul(psv[:], xT[:, kc, rc * 128:(rc + 1) * 128],
                                     wv[:, kc, :], start=kc == 0, stop=kc == 3)
                nc.vector.tensor_scalar_mul(vsb[:, rc, :, 0:64], psv[:],
                                            mk[:, rc:rc + 1])
                nc.vector.tensor_scalar_mul(vsb[:, rc, :, 64:65], ones41[:],
                                            mk[:, rc:rc + 1])
            return emit

        # normalization state per (p, qc)
        ou_tiles = {}
        rec_tiles = {}

        def norm1_unit(p, qc):
            # reciprocal of the denominators via [16, 64] reshape
            def emit():
                ou = ou_tiles[(p, qc)]
                rr = rr_pool.tile([16, 64], bf16, tag="rr")
                nc.sync.dma_start(rr[:], ou[64:65, :])
                rrf = rr_pool.tile([16, 64], f32r, tag="rrf")
                with nc.allow_low_precision(reason="f32r view for PE broadcast"):
                    nc.vector.reciprocal(rrf[:], rr[:])
                rec = rec_pool.tile([1, 1024], f32r, tag="rec")
                nc.sync.dma_start(rec[:], rrf[:])
                rec_tiles[(p, qc)] = rec
            return emit

        def norm2_unit(p, qc):
            # broadcast recips to 64 partitions, then scale o -> oTn2
            def emit():
                qsl = slice(qc * QC, (qc + 1) * QC)
                ou = ou_tiles.pop((p, qc))
                rec = rec_tiles.pop((p, qc))
                bcA = ps_m.tile([64, 512], f32, tag="misc")
                nc.tensor.matmul(bcA[:], ones64r[:], rec[0:1, 0:512],
                                 start=True, stop=True)
                bcB = ps_m.tile([64, 512], f32, tag="misc")
                nc.tensor.matmul(bcB[:], ones64r[:], rec[0:1, 512:1024],
                                 start=True, stop=True)
                bcs = bcs_pool.tile([64, 1024], bf16, tag="bcs")
                nc.vector.tensor_copy(bcs[:, 0:512], bcA[:])
                nc.vector.tensor_copy(bcs[:, 512:1024], bcB[:])
                nc.vector.tensor_mul(oTn2[0:64, p, qsl], ou[0:64, 0:512],
                                     bcs[:, 0:512])
                scrB = work.tile([64, 512], bf16, tag="scrB")
                nc.vector.tensor_mul(scrB[:], ou[0:64, 512:1024], bcs[:, 512:1024])
                # partition shift 0-63 -> 64-127 (DVE lanes are partition-locked)
                nc.sync.dma_start(oTn2[64:128, p, qsl], scrB[:])
            return emit

        def outproj_unit(qc, half):
            # two rsl chunks of 128 queries each
            def emit():
                for rc in (4 * qc + 2 * half, 4 * qc + 2 * half + 1):
                    rsl = slice(rc * 128, (rc + 1) * 128)
                    psy = ps_m.tile([128, 512], f32, tag="misc")
                    for p_ in range(2):
                        nc.tensor.matmul(psy[:], oTn2[:, p_, rsl], wo[:, p_, :],
                                         start=p_ == 0, stop=p_ == 1)
                    yt = yt_pool.tile([128, 512], bf16, tag="yt")
                    nc.vector.tensor_copy(yt[:], psy[:])
                    nc.gpsimd.dma_start(y_d[rsl, :], yt[:])
            return emit

        def drain_fillers(n):
            for _ in range(n):
                if fillers:
                    fillers.pop(0)()

        # ---- lead-in: qkv for chunk 0 --------------------------------------
        qk_unit(0, 0, 0)()
        qk_unit(0, 0, 1)()
        v_unit(0)()
        v_unit(1)()
        fillers.append(v_unit(2))
        fillers.append(v_unit(3))
        fillers.append(qk_unit(0, 1, 0))
        fillers.append(qk_unit(0, 1, 1))

        # filler feed plan: after finishing att(p, qc) push the units that
        # become legal, keyed by (p, qc) just completed.
        feed = {
            (0, 0): [v_unit(4), v_unit(5), v_unit(6), v_unit(7),
                     qk_unit(0, 2, 0), qk_unit(0, 2, 1)],
            (0, 1): [v_unit(8), v_unit(9), v_unit(10), v_unit(11),
                     qk_unit(0, 3, 0), qk_unit(0, 3, 1)],
            (0, 2): [v_unit(12), v_unit(13), v_unit(14), v_unit(15),
                     qk_unit(1, 0, 0), qk_unit(1, 0, 1),
                     qk_unit(1, 1, 0), qk_unit(1, 1, 1)],
            (0, 3): [qk_unit(1, 2, 0), qk_unit(1, 2, 1),
                     qk_unit(1, 3, 0), qk_unit(1, 3, 1)],
            (1, 0): [], (1, 1): [], (1, 2): [], (1, 3): [],
        }

        # ---- attention (pair-major; 2 heads per 2-bank score tile) ----------
        def emit_av(item, oAB, nkb):
            kb, ex, p = item
            nc.tensor.matmul(oAB[0:65, 0:512], vsb[:, kb, 2 * p, :], ex[:, 0:512],
                             start=kb == 0, stop=kb == nkb - 1)
            nc.tensor.matmul(oAB[0:65, 512:1024], vsb[:, kb, 2 * p + 1, :],
                             ex[:, 512:1024], start=kb == 0, stop=kb == nkb - 1)

        for p in range(2):
            for qc in range(NQC):
                nkb = 4 * (qc + 1)
                qsl = slice(qc * QC, (qc + 1) * QC)
                oAB = ps_o.tile([128, 1024], f32, tag="o")
                avq = []
                for kb in range(nkb):
                    ksl = slice(kb * KB, (kb + 1) * KB)
                    sps = ps_sc.tile([128, 1024], f32, tag="scores")
                    # row-tiled pair: K=64 each, concurrent in the array;
                    # outputs land in DIFFERENT PSUM banks (same-bank
                    # dual-write faults the exec unit)
                    nc.tensor.matmul(sps[:, 0:512], kT2[0:64, p, ksl],
                                     qT2[0:64, p, qsl], start=True, stop=True,
                                     tile_position=(0, 0))
                    nc.tensor.matmul(sps[:, 512:1024], kT2[64:128, p, ksl],
                                     qT2[64:128, p, qsl], start=True, stop=True,
                                     tile_position=(64, 0))
                    ex = exp_pool.tile([128, 1024], bf16, tag="ex")
                    nc.scalar.activation(ex[:], sps[:], Exp, scale=SCALE)
                    v_ = kb - (nkb - 4)
                    if v_ >= 0:
                        # causal mask on the restricted (possibly-zero) region
                        w_ = 128 * (v_ + 1)
                        if w_ >= 512:
                            nc.vector.tensor_mul(ex[:], ex[:], dm[:, 3, :])
                        else:
                            nc.vector.tensor_mul(ex[:, 0:w_], ex[:, 0:w_],
                                                 dm[:, v_, 0:w_])
                            nc.vector.tensor_mul(ex[:, 512:512 + w_],
                                                 ex[:, 512:512 + w_],
                                                 dm[:, v_, 512:512 + w_])
                    avq.append((kb, ex, p))
                    if len(avq) > 1:
                        emit_av(avq.pop(0), oAB, nkb)
                    drain_fillers(1)
                emit_av(avq.pop(0), oAB, nkb)

                # eager eviction: one copy frees the PSUM o-tile
                ou = ou_pool.tile([65, 1024], bf16, tag="ou")
                nc.vector.tensor_copy(ou[:], oAB[0:65, :])
                ou_tiles[(p, qc)] = ou

                fillers.extend(feed[(p, qc)])
                fillers.append(norm1_unit(p, qc))
                fillers.append(norm2_unit(p, qc))
                if p == 1:
                    fillers.append(outproj_unit(qc, 0))
                    fillers.append(outproj_unit(qc, 1))

        # ---- tail: whatever fillers remain ----------------------------------
        drain_fillers(len(fillers))

    nc.compile()
    return nc


def _diag_masks():
    i = np.arange(QC)[None, :]
    j = np.arange(KB)[:, None]
    out = []
    for v in range(4):
        mv = np.where(i >= j + v * KB, 1.0, 0.0).astype(np.float32)
        out.append(np.tile(mv, (1, 2)).copy())
    return out


def _prep_inputs(x, m, w_qkv, w_out):
    """Per-core input maps for SPMD dispatch."""
    dm4 = np.stack(_diag_masks()).astype(ml_dtypes.bfloat16)
    wq_full = w_qkv[:, 0:D]
    wk_full = w_qkv[:, D:2 * D]
    wv_full = w_qkv[:, 2 * D:3 * D]
    in_maps = []
    for c in range(8):
        b, q = c // 2, c % 2
        hsl = slice(4 * q * DH, (4 * q + 4) * DH)
        wq2 = np.stack([
            np.concatenate([wq_full[:, (4 * q + 2 * p) * DH:(4 * q + 2 * p + 1) * DH],
                            wq_full[:, (4 * q + 2 * p + 1) * DH:(4 * q + 2 * p + 2) * DH]],
                           axis=1)
            for p in range(2)])
        wk2 = np.stack([
            np.concatenate([wk_full[:, (4 * q + 2 * p) * DH:(4 * q + 2 * p + 1) * DH],
                            wk_full[:, (4 * q + 2 * p + 1) * DH:(4 * q + 2 * p + 2) * DH]],
                           axis=1)
            for p in range(2)])
        in_maps.append({
            "xT": np.ascontiguousarray(x[b].T).astype(ml_dtypes.bfloat16),
            "wq2": np.ascontiguousarray(wq2).astype(ml_dtypes.bfloat16),
            "wk2": np.ascontiguousarray(wk2).astype(ml_dtypes.bfloat16),
            "wv4": np.ascontiguousarray(wv_full[:, hsl]).astype(ml_dtypes.bfloat16),
            "wo4": np.ascontiguousarray(w_out[hsl, :]).astype(ml_dtypes.bfloat16),
            "dm4": dm4,
            "mkey": np.ascontiguousarray((m[b] != 0).astype(np.float32)[:, None]),
        })
    return in_maps


def _execute(inputs, trace=False):
    from concourse.bass_utils import run_bass_kernel_spmd

    if "nc" not in _CACHE:
        _CACHE["nc"] = _build_program()
    nc = _CACHE["nc"]

    x = np.asarray(inputs["x"], np.float32)
    m = np.asarray(inputs["m"], np.float32)
    w_qkv = np.asarray(inputs["w_qkv"], np.float32)
    w_out = np.asarray(inputs["w_out"], np.float32)
    b_out = np.asarray(inputs["b_out"], np.float32)

    in_maps = _prep_inputs(x, m, w_qkv, w_out)
    res = run_bass_kernel_spmd(nc, in_maps, core_ids=list(range(8)), trace=trace)

    y = np.empty((B, T, D), np.float32)
    for b in range(B):
        y[b] = (res.results[2 * b]["y"].astype(np.float32)
                + res.results[2 * b + 1]["y"].astype(np.float32))
    y += b_out[None, None, :]
    y *= m[..., None]
    return y, res


def kernel(**inputs) -> np.ndarray:
    y, _ = _execute(inputs, trace=False)
    return y


# revision 5
# speedup vs baseline: 1.4192x; 1.0567x over previous
"""Trainium2 Bass kernel for nn_DiffusionModel_56822417326086.

Causal multi-head self-attention block:
    qkv = x @ w_qkv ; split into 8 heads of 64
    e = (q @ k^T) * DH^-0.5 ; causal + key-padding mask ; a = softmax(e)
    o = a @ v ; y = o @ w_out + b_out ; y *= m

Sharding (8 cores, zero collectives):
    core c -> batch b = c // 2, head-quad q = c % 2 (heads 4q..4q+3).
    Host sums the two partial output projections per batch, adds b_out,
    applies the query-side mask.

v3: software-pipelined single-pass schedule.  The ACT engine's Exp
stream (~80us on [128,1024] tiles) is the hard floor; qkv projection,
score matmuls, A@V, normalization and output projection are emitted
interleaved between attention blocks (with per-unit drain delays
matched to their dependency latency) so the in-order engine FIFOs
overlap everything under the Exp stream without head-of-line stalls.

On-device layout notes:
  - scores are computed TRANSPOSED: sT[key, query]; keys on partitions
    feed the A@V contraction directly.
  - softmax denominators come free as a 65th "ones" column of V.
  - diagonal key-blocks are RESTRICTED to the causal query range
    (q >= 128*v): smaller score matmuls, per-head exps, [128,128]
    masks, and narrower A@V.
  - o-tiles are evicted eagerly (one [65,1024] copy) so the PSUM o
    pool is a single buffer; normalization runs from SBUF, one qc
    behind attention.
  - reciprocal runs on a DMA-reshaped [16,64] tile (DVE reciprocal is
    8 cyc/elem along the free dim; [1,1024] rows would cost 8.5us).
  - recips are replicated to 64 partitions by gpsimd partition
    broadcast (no PE matmul, no PSUM round-trip).
"""

import numpy as np
import ml_dtypes
from contextlib import ExitStack

B, T, D, H = 4, 2048, 512, 8
DH = D // H
SCALE = DH ** -0.5
QC = 512           # query-chunk (free dim of score matmuls)
NQC = T // QC      # 4
KB = 128           # key-block (partition dim of score tiles)

_CACHE = {}


def _build_program():
    import concourse.mybir as mybir
    import concourse.tile as tile
    from concourse import bacc

    f32 = mybir.dt.float32
    f32r = mybir.dt.float32r
    bf16 = mybir.dt.bfloat16
    Exp = mybir.ActivationFunctionType.Exp

    nc = bacc.Bacc("TRN2", target_bir_lowering=False, debug=False)

    xT_d = nc.dram_tensor("xT", [D, T], bf16, kind="ExternalInput").ap()
    wq_d = nc.dram_tensor("wq2", [2, D, 128], bf16, kind="ExternalInput").ap()
    wk_d = nc.dram_tensor("wk2", [2, D, 128], bf16, kind="ExternalInput").ap()
    wv_d = nc.dram_tensor("wv4", [D, 256], bf16, kind="ExternalInput").ap()
    wo_d = nc.dram_tensor("wo4", [256, D], bf16, kind="ExternalInput").ap()
    dm_d = nc.dram_tensor("dm4", [4, 128, 1024], bf16, kind="ExternalInput").ap()
    mk_d = nc.dram_tensor("mkey", [128, 16], f32, kind="ExternalInput").ap()
    y_d = nc.dram_tensor("y", [T, D], bf16, kind="ExternalOutput").ap()

    with tile.TileContext(nc) as tc, ExitStack() as ctx:
        consts = ctx.enter_context(tc.tile_pool(name="consts", bufs=1))
        ps_sc = ctx.enter_context(tc.tile_pool(name="pssc", bufs=2, space="PSUM"))
        ps_o = ctx.enter_context(tc.tile_pool(name="pso", bufs=1, space="PSUM"))
        ps_m = ctx.enter_context(tc.tile_pool(name="psm", bufs=2, space="PSUM"))
        exp_pool = ctx.enter_context(tc.tile_pool(name="exp", bufs=4))
        work = ctx.enter_context(tc.tile_pool(name="work", bufs=2))
        ou_pool = ctx.enter_context(tc.tile_pool(name="ou", bufs=2))
        rr_pool = ctx.enter_context(tc.tile_pool(name="rr", bufs=2))
        rec_pool = ctx.enter_context(tc.tile_pool(name="rec", bufs=2))
        bcs_pool = ctx.enter_context(tc.tile_pool(name="bcs", bufs=2))
        yt_pool = ctx.enter_context(tc.tile_pool(name="yt", bufs=2))

        # ---- persistent tiles ----------------------------------------------
        # packed q^T/k^T: partitions 0-63 = head A of pair, 64-127 = head B
        qT2 = consts.tile([128, 2, T], f32r)
        kT2 = consts.tile([128, 2, T], f32r)
        vsb = consts.tile([128, 16, 4, 65], bf16)
        wo = consts.tile([128, 2, D], bf16)
        mk = consts.tile([128, 16], f32)
        ones41 = consts.tile([128, 4, 1], f32)
        oTn2 = consts.tile([128, 2, T], bf16)
        ones64 = consts.tile([1, 64], f32)
        ones64r = consts.tile([1, 64], f32r)
        dm = consts.tile([128, 4, 1024], bf16)
        wq = consts.tile([128, 2, 4, 128], bf16)
        wk = consts.tile([128, 2, 4, 128], bf16)
        wv = consts.tile([128, 4, 256], bf16)
        xT = consts.tile([128, 4, T], bf16)
        junk = consts.tile([128, 64], f32)
        junkb = consts.tile([1, 64], bf16)

        nc.vector.memset(ones41[:], 1.0)
        nc.vector.memset(ones64[:], 1.0)
        nc.vector.tensor_copy(ones64r[:], ones64[:])
        nc.vector.memset(junk[:], 0.0)
        nc.vector.memset(junkb[:], 1.0)

        # ---- input DMAs (ordered so the first work's inputs land first) ----
        for kc in range(4):
            nc.scalar.dma_start(wq[:, 0, kc, :], wq_d[0, kc * 128:(kc + 1) * 128, :])
            nc.scalar.dma_start(wk[:, 0, kc, :], wk_d[0, kc * 128:(kc + 1) * 128, :])
        for rc4 in range(4):
            for kc in range(4):
                nc.sync.dma_start(
                    xT[:, kc, rc4 * 512:(rc4 + 1) * 512],
                    xT_d[kc * 128:(kc + 1) * 128, rc4 * 512:(rc4 + 1) * 512])
            if rc4 == 0:
                # v weights + masks right after the first x chunk
                for kc in range(4):
                    nc.sync.dma_start(wv[:, kc, :], wv_d[kc * 128:(kc + 1) * 128, :])
                nc.sync.dma_start(mk[:], mk_d[:])
                for v_ in range(4):
                    nc.sync.dma_start(dm[:, v_, :], dm_d[v_])
            if rc4 == 1:
                for kc in range(4):
                    nc.gpsimd.dma_start(wq[:, 1, kc, :],
                                        wq_d[1, kc * 128:(kc + 1) * 128, :])
                    nc.gpsimd.dma_start(wk[:, 1, kc, :],
                                      wk_d[1, kc * 128:(kc + 1) * 128, :])
            if rc4 == 2:
                for p_ in range(2):
                    nc.gpsimd.dma_start(wo[:, p_, :], wo_d[p_ * 128:(p_ + 1) * 128, :])

        # ---- early ACT table load + gpsimd ucode warm + PE warmup ----------
        wex = exp_pool.tile([128, 64], bf16, tag="ex")
        nc.scalar.activation(wex[:], junk[:], Exp, scale=SCALE)
        wbc = bcs_pool.tile([2, 64], bf16, tag="bcs")
        nc.gpsimd.partition_broadcast(wbc[:], junkb[:])
        warm = consts.tile([1, 512], f32r)
        nc.vector.tensor_copy(warm[0:1, 0:64], ones64[:])
        for _ in range(4):
            wps = ps_m.tile([64, 512], f32, tag="misc")
            nc.tensor.matmul(wps[:], ones64r[:], warm[:], start=True, stop=True)

        # ---- filler machinery: (ready_at_block, emit_fn) -------------------
        fillers = []
        blk = [0]

        def push(fn, delay=0):
            fillers.append([blk[0] + delay, fn])

        def drain(n, force=False):
            done = 0
            for item in list(fillers):
                if done >= n:
                    break
                if force or item[0] <= blk[0]:
                    fillers.remove(item)
                    item[1]()
                    done += 1

        def qk_unit(p, rc4, which):
            def emit():
                sl = slice(rc4 * 512, (rc4 + 1) * 512)
                ps = ps_m.tile([128, 512], f32, tag="misc")
                w_ = wq if which == 0 else wk
                dst = qT2 if which == 0 else kT2
                for kc in range(4):
                    nc.tensor.matmul(ps[:], w_[:, p, kc, :], xT[:, kc, sl],
                                     start=kc == 0, stop=kc == 3)
                nc.vector.tensor_copy(dst[:, p, sl], ps[:])
            return emit

        def v_unit(rc):
            def emit():
                psv = ps_m.tile([128, 4, 64], f32, tag="misc")
                for kc in range(4):
                    nc.tensor.matmul(psv[:], xT[:, kc, rc * 128:(rc + 1) * 128],
                                     wv[:, kc, :], start=kc == 0, stop=kc == 3)
                nc.vector.tensor_scalar_mul(vsb[:, rc, :, 0:64], psv[:],
                                            mk[:, rc:rc + 1])
                nc.vector.tensor_scalar_mul(vsb[:, rc, :, 64:65], ones41[:],
                                            mk[:, rc:rc + 1])
            return emit

        # normalization state per (p, qc)
        ou_tiles = {}
        rec_tiles = {}

        def norm1_unit(p, qc):
            # reciprocal of the denominators via [16, 64] reshape
            def emit():
                ou = ou_tiles[(p, qc)]
                rr = rr_pool.tile([16, 64], bf16, tag="rr")
                nc.sync.dma_start(rr[:], ou[64:65, :])
                rrf = rr_pool.tile([16, 64], bf16, tag="rrf")
                with nc.allow_low_precision(reason="bf16 softmax recip"):
                    nc.vector.reciprocal(rrf[:], rr[:])
                rec = rec_pool.tile([1, 1024], bf16, tag="rec")
                nc.sync.dma_start(rec[:], rrf[:])
                rec_tiles[(p, qc)] = rec
            return emit

        def norm2_unit(p, qc):
            # replicate recips to 64 partitions, then scale o -> oTn2
            def emit():
                qsl = slice(qc * QC, (qc + 1) * QC)
                ou = ou_tiles.pop((p, qc))
                rec = rec_tiles.pop((p, qc))
                bcs = bcs_pool.tile([64, 1024], bf16, tag="bcs")
                nc.gpsimd.partition_broadcast(bcs[:], rec[:])
                nc.vector.tensor_mul(oTn2[0:64, p, qsl], ou[0:64, 0:512],
                                     bcs[:, 0:512])
                scrB = work.tile([64, 512], bf16, tag="scrB")
                nc.vector.tensor_mul(scrB[:], ou[0:64, 512:1024], bcs[:, 512:1024])
                # partition shift 0-63 -> 64-127 (DVE lanes are partition-locked)
                nc.sync.dma_start(oTn2[64:128, p, qsl], scrB[:])
            return emit

        def outproj_unit(qc, half):
            # two rsl chunks of 128 queries each
            def emit():
                for rc in (4 * qc + 2 * half, 4 * qc + 2 * half + 1):
                    rsl = slice(rc * 128, (rc + 1) * 128)
                    psy = ps_m.tile([128, 512], f32, tag="misc")
                    for p_ in range(2):
                        nc.tensor.matmul(psy[:], oTn2[:, p_, rsl], wo[:, p_, :],
                                         start=p_ == 0, stop=p_ == 1)
                    yt = yt_pool.tile([128, 512], bf16, tag="yt")
                    nc.vector.tensor_copy(yt[:], psy[:])
                    nc.gpsimd.dma_start(y_d[rsl, :], yt[:])
            return emit

        # ---- lead-in: qkv for chunk 0 --------------------------------------
        qk_unit(0, 0, 0)()
        qk_unit(0, 0, 1)()
        v_unit(0)()
        v_unit(1)()
        push(v_unit(2))
        push(v_unit(3))
        push(qk_unit(0, 1, 0))
        push(qk_unit(0, 1, 1))

        feed = {
            (0, 0): [v_unit(4), v_unit(5), v_unit(6), v_unit(7),
                     qk_unit(0, 2, 0), qk_unit(0, 2, 1)],
            (0, 1): [v_unit(8), v_unit(9), v_unit(10), v_unit(11),
                     qk_unit(0, 3, 0), qk_unit(0, 3, 1)],
            (0, 2): [v_unit(12), v_unit(13), v_unit(14), v_unit(15),
                     qk_unit(1, 0, 0), qk_unit(1, 0, 1),
                     qk_unit(1, 1, 0), qk_unit(1, 1, 1)],
            (0, 3): [qk_unit(1, 2, 0), qk_unit(1, 2, 1),
                     qk_unit(1, 3, 0), qk_unit(1, 3, 1)],
            (1, 0): [], (1, 1): [], (1, 2): [], (1, 3): [],
        }

        # ---- attention (pair-major; 2 heads per 2-bank score tile) ----------
        def emit_av(item, oAB, nkb):
            kb, ex, p, qoff = item
            nc.tensor.matmul(oAB[0:65, qoff:512], vsb[:, kb, 2 * p, :],
                             ex[:, qoff:512], start=kb == 0, stop=kb == nkb - 1)
            nc.tensor.matmul(oAB[0:65, 512 + qoff:1024], vsb[:, kb, 2 * p + 1, :],
                             ex[:, 512 + qoff:1024],
                             start=kb == 0, stop=kb == nkb - 1)

        for p in range(2):
            for qc in range(NQC):
                nkb = 4 * (qc + 1)
                oAB = ps_o.tile([128, 1024], f32, tag="o")
                avq = []
                for kb in range(nkb):
                    ksl = slice(kb * KB, (kb + 1) * KB)
                    v_ = kb - (nkb - 4)
                    qoff = 128 * v_ if v_ > 0 else 0
                    qsl = slice(qc * QC + qoff, (qc + 1) * QC)
                    sps = ps_sc.tile([128, 1024], f32, tag="scores")
                    # row-tiled pair: K=64 each, concurrent in the array;
                    # outputs land in DIFFERENT PSUM banks (same-bank
                    # dual-write faults the exec unit)
                    nc.tensor.matmul(sps[:, qoff:512], kT2[0:64, p, ksl],
                                     qT2[0:64, p, qsl], start=True, stop=True,
                                     tile_position=(0, 0))
                    nc.tensor.matmul(sps[:, 512 + qoff:1024], kT2[64:128, p, ksl],
                                     qT2[64:128, p, qsl], start=True, stop=True,
                                     tile_position=(64, 0))
                    ex = exp_pool.tile([128, 1024], bf16, tag="ex")
                    if qoff:
                        nc.scalar.activation(ex[:, qoff:512], sps[:, qoff:512],
                                             Exp, scale=SCALE)
                        nc.scalar.activation(ex[:, 512 + qoff:1024],
                                             sps[:, 512 + qoff:1024],
                                             Exp, scale=SCALE)
                    else:
                        nc.scalar.activation(ex[:], sps[:], Exp, scale=SCALE)
                    if v_ >= 0:
                        # causal mask on the in-block triangle only
                        nc.vector.tensor_mul(ex[:, qoff:qoff + 128],
                                             ex[:, qoff:qoff + 128],
                                             dm[:, v_, qoff:qoff + 128])
                        nc.vector.tensor_mul(ex[:, 512 + qoff:512 + qoff + 128],
                                             ex[:, 512 + qoff:512 + qoff + 128],
                                             dm[:, v_, 512 + qoff:512 + qoff + 128])
                    avq.append((kb, ex, p, qoff))
                    if len(avq) > 1:
                        emit_av(avq.pop(0), oAB, nkb)
                    drain(1)
                    blk[0] += 1
                emit_av(avq.pop(0), oAB, nkb)

                # eager eviction: one copy frees the PSUM o-tile
                ou = ou_pool.tile([65, 1024], bf16, tag="ou")
                nc.vector.tensor_copy(ou[:], oAB[0:65, :])
                ou_tiles[(p, qc)] = ou

                for fn in feed[(p, qc)]:
                    push(fn)
                push(norm1_unit(p, qc), delay=2)
                push(norm2_unit(p, qc), delay=5)
                if p == 1:
                    push(outproj_unit(qc, 0), delay=8)
                    push(outproj_unit(qc, 1), delay=9)

        # ---- tail: whatever fillers remain ----------------------------------
        drain(len(fillers), force=True)

    nc.compile()
    return nc


def _diag_masks():
    i = np.arange(QC)[None, :]
    j = np.arange(KB)[:, None]
    out = []
    for v in range(4):
        mv = np.where(i >= j + v * KB, 1.0, 0.0).astype(np.float32)
        out.append(np.tile(mv, (1, 2)).copy())
    return out


def _prep_inputs(x, m, w_qkv, w_out):
    """Per-core input maps for SPMD dispatch."""
    dm4 = np.stack(_diag_masks()).astype(ml_dtypes.bfloat16)
    wq_full = w_qkv[:, 0:D]
    wk_full = w_qkv[:, D:2 * D]
    wv_full = w_qkv[:, 2 * D:3 * D]
    in_maps = []
    for c in range(8):
        b, q = c // 2, c % 2
        hsl = slice(4 * q * DH, (4 * q + 4) * DH)
        wq2 = np.stack([
            np.concatenate([wq_full[:, (4 * q + 2 * p) * DH:(4 * q + 2 * p + 1) * DH],
                            wq_full[:, (4 * q + 2 * p + 1) * DH:(4 * q + 2 * p + 2) * DH]],
                           axis=1)
            for p in range(2)])
        wk2 = np.stack([
            np.concatenate([wk_full[:, (4 * q + 2 * p) * DH:(4 * q + 2 * p + 1) * DH],
                            wk_full[:, (4 * q + 2 * p + 1) * DH:(4 * q + 2 * p + 2) * DH]],
                           axis=1)
            for p in range(2)])
        mkey = (m[b] != 0).astype(np.float32)  # [T]
        in_maps.append({
            "xT": np.ascontiguousarray(x[b].T).astype(ml_dtypes.bfloat16),
            "wq2": np.ascontiguousarray(wq2).astype(ml_dtypes.bfloat16),
            "wk2": np.ascontiguousarray(wk2).astype(ml_dtypes.bfloat16),
            "wv4": np.ascontiguousarray(wv_full[:, hsl]).astype(ml_dtypes.bfloat16),
            "wo4": np.ascontiguousarray(w_out[hsl, :]).astype(ml_dtypes.bfloat16),
            "dm4": dm4,
            # [128, 16] partition-major: mkey[p, c] = m[c*128 + p]
            "mkey": np.ascontiguousarray(mkey.reshape(16, 128).T),
        })
    return in_maps


def _execute(inputs, trace=False):
    from concourse.bass_utils import run_bass_kernel_spmd

    if "nc" not in _CACHE:
        _CACHE["nc"] = _build_program()
    nc = _CACHE["nc"]

    x = np.asarray(inputs["x"], np.float32)
    m = np.asarray(inputs["m"], np.float32)
    w_qkv = np.asarray(inputs["w_qkv"], np.float32)
    w_out = np.asarray(inputs["w_out"], np.float32)
    b_out = np.asarray(inputs["b_out"], np.float32)

    in_maps = _prep_inputs(x, m, w_qkv, w_out)
    res = run_bass_kernel_spmd(nc, in_maps, core_ids=list(range(8)), trace=trace)

    y = np.empty((B, T, D), np.float32)
    for b in range(B):
        y[b] = (res.results[2 * b]["y"].astype(np.float32)
                + res.results[2 * b + 1]["y"].astype(np.float32))
    y += b_out[None, None, :]
    y *= m[..., None]
    return y, res


def kernel(**inputs) -> np.ndarray:
    y, _ = _execute(inputs, trace=False)
    return y


# revision 8
# speedup vs baseline: 1.5010x; 1.0576x over previous
"""Trainium2 Bass kernel for nn_DiffusionModel_56822417326086.

Causal multi-head self-attention block:
    qkv = x @ w_qkv ; split into 8 heads of 64
    e = (q @ k^T) * DH^-0.5 ; causal + key-padding mask ; a = softmax(e)
    o = a @ v ; y = o @ w_out + b_out ; y *= m

Sharding (8 cores, zero collectives):
    core c -> batch b = c // 2, head-quad q = c % 2 (heads 4q..4q+3).
    Host sums the two partial output projections per batch, adds b_out,
    applies the query-side mask.

v4: software-pipelined single-pass schedule.  The ACT engine's Exp
stream (~78us) and the PE matmul stream (~76us) are co-critical;
qkv projection, A@V, normalization and output projection are emitted
interleaved between attention blocks (with per-unit drain delays
matched to dependency latency, and dependency-keyed force drains for
correct ordering) so the in-order engine FIFOs overlap everything.

Load-bearing device facts (measured on this HW):
  - fp32/f32r MOVING operands stream at HALF rate (512-col matmul:
    427ns vs 216ns bf16) -> q^T/k^T are kept in bf16.
  - LDWEIGHTS hides under the previous matmul (dual weight buffer) as
    long as matmuls are back-to-back; exposed only after pipeline
    bubbles.
  - DVE reciprocal is 8 cyc/elem along the FREE dim -> reshape sums to
    [16,64] via DMA before reciprocal.
  - ACT per-op overhead ~150-250ns -> full-tile [128,1024] exps even
    for partially-stale diagonal blocks (stale region provably unread)
    except v=3 (two [128,128] exps win).
  - DMA cannot touch PSUM; all PSUM eviction is DVE.
"""

import numpy as np
import ml_dtypes
from contextlib import ExitStack

B, T, D, H = 4, 2048, 512, 8
DH = D // H
SCALE = DH ** -0.5
QC = 512           # query-chunk (free dim of score matmuls)
NQC = T // QC      # 4
KB = 128           # key-block (partition dim of score tiles)

_CACHE = {}


def _build_program():
    import concourse.mybir as mybir
    import concourse.tile as tile
    from concourse import bacc

    f32 = mybir.dt.float32
    f32r = mybir.dt.float32r
    bf16 = mybir.dt.bfloat16
    Exp = mybir.ActivationFunctionType.Exp

    nc = bacc.Bacc("TRN2", target_bir_lowering=False, debug=False)

    xT_d = nc.dram_tensor("xT", [D, T], bf16, kind="ExternalInput").ap()
    wq_d = nc.dram_tensor("wq2", [2, D, 128], bf16, kind="ExternalInput").ap()
    wk_d = nc.dram_tensor("wk2", [2, D, 128], bf16, kind="ExternalInput").ap()
    wv_d = nc.dram_tensor("wv4", [D, 256], bf16, kind="ExternalInput").ap()
    wo_d = nc.dram_tensor("wo4", [256, D], bf16, kind="ExternalInput").ap()
    dm_d = nc.dram_tensor("dm4", [4, 128, 1024], bf16, kind="ExternalInput").ap()
    mk_d = nc.dram_tensor("mkey", [128, 16], f32, kind="ExternalInput").ap()
    y_d = nc.dram_tensor("y", [T, D], bf16, kind="ExternalOutput").ap()

    with tile.TileContext(nc) as tc, ExitStack() as ctx:
        consts = ctx.enter_context(tc.tile_pool(name="consts", bufs=1))
        ps_sc = ctx.enter_context(tc.tile_pool(name="pssc", bufs=2, space="PSUM"))
        ps_o = ctx.enter_context(tc.tile_pool(name="pso", bufs=1, space="PSUM"))
        ps_m = ctx.enter_context(tc.tile_pool(name="psm", bufs=2, space="PSUM"))
        exp_pool = ctx.enter_context(tc.tile_pool(name="exp", bufs=4))
        work = ctx.enter_context(tc.tile_pool(name="work", bufs=2))
        ou_pool = ctx.enter_context(tc.tile_pool(name="ou", bufs=2))
        rr_pool = ctx.enter_context(tc.tile_pool(name="rr", bufs=2))
        rec_pool = ctx.enter_context(tc.tile_pool(name="rec", bufs=2))
        bcs_pool = ctx.enter_context(tc.tile_pool(name="bcs", bufs=2))
        yt_pool = ctx.enter_context(tc.tile_pool(name="yt", bufs=2))

        # ---- persistent tiles ----------------------------------------------
        # packed q^T/k^T: partitions 0-63 = head A of pair, 64-127 = head B
        qT2 = consts.tile([128, 2, T], bf16)
        kT2 = consts.tile([128, 2, T], bf16)
        vsb = consts.tile([128, 16, 4, 65], bf16)
        wo = consts.tile([128, 2, D], bf16)
        mk = consts.tile([128, 16], f32)
        ones41 = consts.tile([128, 4, 1], f32)
        oTn2 = consts.tile([128, 2, T], bf16)
        ones64 = consts.tile([1, 64], f32)
        ones64r = consts.tile([1, 64], f32r)
        dm = consts.tile([128, 4, 1024], bf16)
        wq = consts.tile([128, 2, 4, 128], bf16)
        wk = consts.tile([128, 2, 4, 128], bf16)
        wv = consts.tile([128, 4, 256], bf16)
        xT = consts.tile([128, 4, T], bf16)
        junk = consts.tile([128, 64], f32)
        junkb = consts.tile([1, 64], bf16)

        nc.vector.memset(junk[:], 0.0)
        nc.gpsimd.memset(junkb[:], 1.0)

        # early ACT table load + gpsimd ucode warm
        wex = exp_pool.tile([128, 64], bf16, tag="ex")
        nc.scalar.activation(wex[:], junk[:], Exp, scale=SCALE)
        wbc = bcs_pool.tile([2, 64], bf16, tag="bcs")
        nc.gpsimd.partition_broadcast(wbc[:], junkb[:])

        nc.vector.memset(ones41[:], 1.0)
        nc.vector.memset(ones64[:], 1.0)
        nc.vector.tensor_copy(ones64r[:], ones64[:])

        # ---- input DMAs (ordered so the first work's inputs land first) ----
        for kc in range(4):
            nc.scalar.dma_start(wq[:, 0, kc, :], wq_d[0, kc * 128:(kc + 1) * 128, :])
            nc.scalar.dma_start(wk[:, 0, kc, :], wk_d[0, kc * 128:(kc + 1) * 128, :])
        for rc4 in range(4):
            for kc in range(4):
                nc.sync.dma_start(
                    xT[:, kc, rc4 * 512:(rc4 + 1) * 512],
                    xT_d[kc * 128:(kc + 1) * 128, rc4 * 512:(rc4 + 1) * 512])
            if rc4 == 0:
                # v weights + masks right after the first x chunk
                for kc in range(4):
                    nc.sync.dma_start(wv[:, kc, :], wv_d[kc * 128:(kc + 1) * 128, :])
                nc.sync.dma_start(mk[:], mk_d[:])
                for v_ in range(4):
                    nc.sync.dma_start(dm[:, v_, :], dm_d[v_])
            if rc4 == 1:
                for kc in range(4):
                    nc.gpsimd.dma_start(wq[:, 1, kc, :],
                                        wq_d[1, kc * 128:(kc + 1) * 128, :])
                    nc.gpsimd.dma_start(wk[:, 1, kc, :],
                                        wk_d[1, kc * 128:(kc + 1) * 128, :])
            if rc4 == 2:
                for p_ in range(2):
                    nc.gpsimd.dma_start(wo[:, p_, :], wo_d[p_ * 128:(p_ + 1) * 128, :])

        # ---- PE warmup ------------------------------------------------------
        warm = consts.tile([1, 512], f32r)
        nc.vector.tensor_copy(warm[0:1, 0:64], ones64[:])

        def warm_mm():
            wps = ps_m.tile([64, 512], f32, tag="misc")
            nc.tensor.matmul(wps[:], ones64r[:], warm[:], start=True, stop=True)

        for _ in range(4):
            warm_mm()

        # ---- filler machinery: [ready_at_block, key, emit_fn] --------------
        fillers = []
        blk = [0]

        def push(fn, delay=0, key=None):
            fillers.append([blk[0] + delay, key, fn])

        def drain(n, force=False):
            done = 0
            for item in list(fillers):
                if done >= n:
                    break
                if force or item[0] <= blk[0]:
                    fillers.remove(item)
                    item[2]()
                    done += 1

        def need(key):
            for item in list(fillers):
                if item[1] == key:
                    fillers.remove(item)
                    item[2]()
                    return

        def qk_unit(p, rc4, which):
            def emit():
                sl = slice(rc4 * 512, (rc4 + 1) * 512)
                ps = ps_m.tile([128, 512], f32, tag="misc")
                w_ = wq if which == 0 else wk
                dst = qT2 if which == 0 else kT2
                for kc in range(4):
                    nc.tensor.matmul(ps[:], w_[:, p, kc, :], xT[:, kc, sl],
                                     start=kc == 0, stop=kc == 3)
                nc.vector.tensor_copy(dst[:, p, sl], ps[:])
            return emit

        def v_unit(rc):
            def emit():
                psv = ps_m.tile([128, 4, 64], f32, tag="misc")
                for kc in range(4):
                    nc.tensor.matmul(psv[:], xT[:, kc, rc * 128:(rc + 1) * 128],
                                     wv[:, kc, :], start=kc == 0, stop=kc == 3)
                nc.vector.tensor_scalar_mul(vsb[:, rc, :, 0:64], psv[:],
                                            mk[:, rc:rc + 1])
                nc.vector.tensor_scalar_mul(vsb[:, rc, :, 64:65], ones41[:],
                                            mk[:, rc:rc + 1])
            return emit

        # normalization state per (p, qc)
        ou_tiles = {}
        srow_tiles = {}
        rec_tiles = {}

        def norm1_unit(p, qc):
            # reciprocal of the denominators via [16, 64] reshape
            def emit():
                srow = srow_tiles[(p, qc)]
                rr = rr_pool.tile([16, 64], bf16, tag="rr")
                nc.sync.dma_start(rr[:], srow[:])
                rrf = rr_pool.tile([16, 64], bf16, tag="rrf")
                with nc.allow_low_precision(reason="bf16 softmax recip"):
                    nc.vector.reciprocal(rrf[:], rr[:])
                rec = rec_pool.tile([1, 1024], bf16, tag="rec")
                nc.sync.dma_start(rec[:], rrf[:])
                rec_tiles[(p, qc)] = rec
            return emit

        def norm2_unit(p, qc):
            # replicate recips to 64 partitions, then scale o -> oTn2
            def emit():
                qsl = slice(qc * QC, (qc + 1) * QC)
                ou = ou_tiles.pop((p, qc))
                srow_tiles.pop((p, qc))
                rec = rec_tiles.pop((p, qc))
                bcs = bcs_pool.tile([64, 1024], bf16, tag="bcs")
                nc.gpsimd.partition_broadcast(bcs[:], rec[:])
                nc.vector.tensor_mul(oTn2[0:64, p, qsl], ou[0:64, 0:512],
                                     bcs[:, 0:512])
                scrB = work.tile([64, 512], bf16, tag="scrB")
                nc.vector.tensor_mul(scrB[:], ou[0:64, 512:1024], bcs[:, 512:1024])
                # partition shift 0-63 -> 64-127 (DVE lanes are partition-locked)
                nc.sync.dma_start(oTn2[64:128, p, qsl], scrB[:])
            return emit

        def outproj_unit(qc, half):
            # two rsl chunks of 128 queries each
            def emit():
                for rc in (4 * qc + 2 * half, 4 * qc + 2 * half + 1):
                    rsl = slice(rc * 128, (rc + 1) * 128)
                    psy = ps_m.tile([128, 512], f32, tag="misc")
                    for p_ in range(2):
                        nc.tensor.matmul(psy[:], oTn2[:, p_, rsl], wo[:, p_, :],
                                         start=p_ == 0, stop=p_ == 1)
                    yt = yt_pool.tile([128, 512], bf16, tag="yt")
                    nc.vector.tensor_copy(yt[:], psy[:])
                    nc.gpsimd.dma_start(y_d[rsl, :], yt[:])
            return emit

        # ---- lead-in: q/k for chunk 0; v units drain inside block 0-3 ------
        qk_unit(0, 0, 0)()
        qk_unit(0, 0, 1)()
        for rc in range(4):
            push(v_unit(rc), key=("v", rc))
        push(qk_unit(0, 1, 0), key=("qk", 0, 1, 0))
        push(qk_unit(0, 1, 1), key=("qk", 0, 1, 1))

        feed = {
            (0, 0): [(v_unit(4), ("v", 4)), (v_unit(5), ("v", 5)),
                     (v_unit(6), ("v", 6)), (v_unit(7), ("v", 7)),
                     (qk_unit(0, 2, 0), ("qk", 0, 2, 0)),
                     (qk_unit(0, 2, 1), ("qk", 0, 2, 1))],
            (0, 1): [(v_unit(8), ("v", 8)), (v_unit(9), ("v", 9)),
                     (v_unit(10), ("v", 10)), (v_unit(11), ("v", 11)),
                     (qk_unit(0, 3, 0), ("qk", 0, 3, 0)),
                     (qk_unit(0, 3, 1), ("qk", 0, 3, 1))],
            (0, 2): [(v_unit(12), ("v", 12)), (v_unit(13), ("v", 13)),
                     (v_unit(14), ("v", 14)), (v_unit(15), ("v", 15)),
                     (qk_unit(1, 0, 0), ("qk", 1, 0, 0)),
                     (qk_unit(1, 0, 1), ("qk", 1, 0, 1)),
                     (qk_unit(1, 1, 0), ("qk", 1, 1, 0)),
                     (qk_unit(1, 1, 1), ("qk", 1, 1, 1))],
            (0, 3): [(qk_unit(1, 2, 0), ("qk", 1, 2, 0)),
                     (qk_unit(1, 2, 1), ("qk", 1, 2, 1)),
                     (qk_unit(1, 3, 0), ("qk", 1, 3, 0)),
                     (qk_unit(1, 3, 1), ("qk", 1, 3, 1))],
            (1, 0): [], (1, 1): [], (1, 2): [], (1, 3): [],
        }

        # ---- attention (pair-major; 2 heads per 2-bank score tile) ----------
        def emit_av(item, oAB, nkb):
            kb, ex, p, qoff = item
            need(("v", kb))
            nc.tensor.matmul(oAB[0:65, qoff:512], vsb[:, kb, 2 * p, :],
                             ex[:, qoff:512], start=kb == 0, stop=kb == nkb - 1)
            nc.tensor.matmul(oAB[0:65, 512 + qoff:1024], vsb[:, kb, 2 * p + 1, :],
                             ex[:, 512 + qoff:1024],
                             start=kb == 0, stop=kb == nkb - 1)

        for p in range(2):
            for qc in range(NQC):
                nkb = 4 * (qc + 1)
                # force any q/k production this chunk depends on
                need(("qk", p, qc, 0))
                for r in range(qc + 1):
                    need(("qk", p, r, 1))
                oAB = ps_o.tile([128, 1024], f32, tag="o")
                avq = []
                for kb in range(nkb):
                    ksl = slice(kb * KB, (kb + 1) * KB)
                    v_ = kb - (nkb - 4)
                    qoff = 128 * v_ if v_ > 0 else 0
                    qsl = slice(qc * QC + qoff, (qc + 1) * QC)
                    sps = ps_sc.tile([128, 1024], f32, tag="scores")
                    # row-tiled pair: K=64 each, concurrent in the array;
                    # outputs land in DIFFERENT PSUM banks (same-bank
                    # dual-write faults the exec unit)
                    nc.tensor.matmul(sps[:, qoff:512], kT2[0:64, p, ksl],
                                     qT2[0:64, p, qsl], start=True, stop=True,
                                     tile_position=(0, 0))
                    nc.tensor.matmul(sps[:, 512 + qoff:1024], kT2[64:128, p, ksl],
                                     qT2[64:128, p, qsl], start=True, stop=True,
                                     tile_position=(64, 0))
                    ex = exp_pool.tile([128, 1024], bf16, tag="ex")
                    if v_ == 4 - 1 and qoff:
                        # v=3: two small exps beat one full tile
                        nc.scalar.activation(ex[:, qoff:512], sps[:, qoff:512],
                                             Exp, scale=SCALE)
                        nc.scalar.activation(ex[:, 512 + qoff:1024],
                                             sps[:, 512 + qoff:1024],
                                             Exp, scale=SCALE)
                    else:
                        # full tile; any stale sub-region is never read
                        nc.scalar.activation(ex[:], sps[:], Exp, scale=SCALE)
                    if v_ >= 0:
                        # causal mask on the in-block triangle only
                        nc.vector.tensor_mul(ex[:, qoff:qoff + 128],
                                             ex[:, qoff:qoff + 128],
                                             dm[:, v_, qoff:qoff + 128])
                        nc.vector.tensor_mul(ex[:, 512 + qoff:512 + qoff + 128],
                                             ex[:, 512 + qoff:512 + qoff + 128],
                                             dm[:, v_, 512 + qoff:512 + qoff + 128])
                    avq.append((kb, ex, p, qoff))
                    if len(avq) > 1:
                        emit_av(avq.pop(0), oAB, nkb)
                    drain(1)
                    blk[0] += 1
                emit_av(avq.pop(0), oAB, nkb)

                # eager eviction; sums row first so the recip chain starts early
                srow = rr_pool.tile([1, 1024], bf16, tag="srow")
                nc.vector.tensor_copy(srow[:], oAB[64:65, :])
                ou = ou_pool.tile([64, 1024], bf16, tag="ou")
                nc.vector.tensor_copy(ou[:], oAB[0:64, :])
                srow_tiles[(p, qc)] = srow
                ou_tiles[(p, qc)] = ou

                for fn, key in feed[(p, qc)]:
                    push(fn, key=key)
                push(norm1_unit(p, qc), delay=2)
                push(norm2_unit(p, qc), delay=5)
                if p == 1:
                    push(outproj_unit(qc, 0), delay=8)
                    push(outproj_unit(qc, 1), delay=9)

        # ---- tail: drain remaining units, keeping the PE warm ---------------
        while fillers:
            drain(1, force=True)
            warm_mm()

    nc.compile()
    return nc


def _diag_masks():
    i = np.arange(QC)[None, :]
    j = np.arange(KB)[:, None]
    out = []
    for v in range(4):
        mv = np.where(i >= j + v * KB, 1.0, 0.0).astype(np.float32)
        out.append(np.tile(mv, (1, 2)).copy())
    return out


def _prep_inputs(x, m, w_qkv, w_out):
    """Per-core input maps for SPMD dispatch."""
    dm4 = np.stack(_diag_masks()).astype(ml_dtypes.bfloat16)
    wq_full = w_qkv[:, 0:D]
    wk_full = w_qkv[:, D:2 * D]
    wv_full = w_qkv[:, 2 * D:3 * D]
    in_maps = []
    for c in range(8):
        b, q = c // 2, c % 2
        hsl = slice(4 * q * DH, (4 * q + 4) * DH)
        wq2 = np.stack([
            np.concatenate([wq_full[:, (4 * q + 2 * p) * DH:(4 * q + 2 * p + 1) * DH],
                            wq_full[:, (4 * q + 2 * p + 1) * DH:(4 * q + 2 * p + 2) * DH]],
                           axis=1)
            for p in range(2)])
        wk2 = np.stack([
            np.concatenate([wk_full[:, (4 * q + 2 * p) * DH:(4 * q + 2 * p + 1) * DH],
                            wk_full[:, (4 * q + 2 * p + 1) * DH:(4 * q + 2 * p + 2) * DH]],
                           axis=1)
            for p in range(2)])
        mkey = (m[b] != 0).astype(np.float32)  # [T]
        in_maps.append({
            "xT": np.ascontiguousarray(x[b].T).astype(ml_dtypes.bfloat16),
            "wq2": np.ascontiguousarray(wq2).astype(ml_dtypes.bfloat16),
            "wk2": np.ascontiguousarray(wk2).astype(ml_dtypes.bfloat16),
            "wv4": np.ascontiguousarray(wv_full[:, hsl]).astype(ml_dtypes.bfloat16),
            "wo4": np.ascontiguousarray(w_out[hsl, :]).astype(ml_dtypes.bfloat16),
            "dm4": dm4,
            # [128, 16] partition-major: mkey[p, c] = m[c*128 + p]
            "mkey": np.ascontiguousarray(mkey.reshape(16, 128).T),
        })
    return in_maps


def _execute(inputs, trace=False):
    from concourse.bass_utils import run_bass_kernel_spmd

    if "nc" not in _CACHE:
        _CACHE["nc"] = _build_program()
    nc = _CACHE["nc"]

    x = np.asarray(inputs["x"], np.float32)
    m = np.asarray(inputs["m"], np.float32)
    w_qkv = np.asarray(inputs["w_qkv"], np.float32)
    w_out = np.asarray(inputs["w_out"], np.float32)
    b_out = np.asarray(inputs["b_out"], np.float32)

    in_maps = _prep_inputs(x, m, w_qkv, w_out)
    res = run_bass_kernel_spmd(nc, in_maps, core_ids=list(range(8)), trace=trace)

    y = np.empty((B, T, D), np.float32)
    for b in range(B):
        y[b] = (res.results[2 * b]["y"].astype(np.float32)
                + res.results[2 * b + 1]["y"].astype(np.float32))
    y += b_out[None, None, :]
    y *= m[..., None]
    return y, res


def kernel(**inputs) -> np.ndarray:
    y, _ = _execute(inputs, trace=False)
    return y
